# revision 1
# baseline (speedup 1.0000x reference)
"""Trainium2 Bass kernel for nn_CustomTransformer2D (gnn message passing).

Math (validated against the reference in fp64):
  - The q/LN1/Wq branch only shifts attention logits by a constant along the
    softmax axis M, so it cancels in softmax; q enters only via the residual.
  - bk and bp are constant along M too -> dropped from the logits.
  - v = Wv@(Wk@k+bk)+bv = k@(Wv@Wk)^T + bkv;  since sum_m a = 1, the bkv
    offset commutes through the attention sum and is folded into Wo's bias.
  - ln2 gain/bias are folded into W1/b1.

Per-core layout (batch b on core b): tokens on SBUF partitions; the matmul
path runs in bf16 (inputs are cast during the DMA), accumulation and the
softmax/normalization arithmetic stay fp32.

For each chunk of 128 tokens:
  k-tile [128t, 16m*64d] --PE transpose per m-pair--> T_j [128=(r,d), 128t]
  S/v matmul: lhsT=T_j, rhs=Wsv[128=(r,din), 256=(Se|So|ve|vo)] -> PSUM f32
  pos matmul: lhsT=posT[64=(m,c),128t], rhs=bank-padded Wp -> accumulate S
  softmax over m in free dims: exp on ACT (exp/copy/relu share one ACT
  table; no Sqrt => no table reloads), e*v and grouped reduces on DVE in a
  (d, m)-contiguous layout, rstd via Quake rsqrt + Newton on DVE.
  Tail (Wo, LN2, MLP) in natural token layout, biases injected via a
  preloaded ones-row in the transposed activations.
"""

import numpy as np

B, N, M, D, F = 8, 16384, 16, 64, 256
EPS = 1e-5
CHUNK = 128  # tokens per chunk
RSQRT_MAGIC = 0x5F3759DF

_prog_cache = {}


def _bf16(x):
    import ml_dtypes

    return np.asarray(x).astype(ml_dtypes.bfloat16)


def _prep_weights(inp):
    f = np.float32
    Wk, bk = inp["Wk"], inp["bk"]
    Wv, bv = inp["Wv"], inp["bv"]
    Wp = inp["Wp"]
    Wo, bo = inp["Wo"], inp["bo"]
    W1, b1 = inp["W1"], inp["b1"]
    W2, b2 = inp["W2"], inp["b2"]
    g2, bl2 = inp["ln2_g"], inp["ln2_b"]

    Wkv = (Wv @ Wk).astype(f)
    bkv = (Wv @ bk + bv).astype(f)
    bo_p = (bo + Wo @ bkv).astype(f)
    W1p = (W1 * g2[None, :]).astype(f)
    b1p = (b1 + W1 @ bl2).astype(f)

    wsv = np.zeros((128, 256), f)
    wsv[0:64, 0:64] = Wk.T
    wsv[64:128, 64:128] = Wk.T
    wsv[0:64, 128:192] = Wkv.T
    wsv[64:128, 192:256] = Wkv.T

    # pos weights padded to full PSUM banks: col = 256*j + 64*r + d for
    # m = 2j+r; v columns (256*j+128..256*j+255) stay zero.
    wpos = np.zeros((64, 2048), f)
    for m in range(16):
        j, r = m // 2, m % 2
        wpos[4 * m : 4 * m + 4, 256 * j + 64 * r : 256 * j + 64 * r + 64] = Wp.T

    woe = np.zeros((65, 64), f)
    woe[0:64] = Wo.T
    woe[64] = bo_p

    w1e = np.zeros((65, 256), f)
    w1e[0:64] = W1p.T
    w1e[64] = b1p

    w2s = np.concatenate([W2.T[0:128], W2.T[128:256]], axis=1).astype(f)  # [128,128]

    return {
        "wsv": _bf16(wsv),
        "wpos": _bf16(wpos),
        "woe": _bf16(woe),
        "w1e": w1e,
        "w2s": w2s,
        "b2c": b2.astype(f),
        "ident": _bf16(np.eye(128, dtype=f)),
        "ones": _bf16(np.ones(128, f)),
        "ones32": np.ones(128, f),
        "ident32": np.eye(128, dtype=f),
    }


def _patch_tile_drain():
    """This container's walrus build rejects instructions with more than one
    sync-wait command. Tile's kernel-tail drain carries one wait per logical
    processor; split them across sync-engine nops."""
    import concourse.tile as tile
    from concourse.vector_clock import ScopedClock, VectorClock

    if getattr(tile.TileContext, "_ant_drain_patched", False):
        return

    def _drain_and_barrier(self, tick_clock, wait_clock):
        nc = self.nc
        gc = tick_clock.global_clock
        n = len(gc)
        procs = [i for i in range(n) if gc[i] > 0]
        for p in procs:
            sub = VectorClock([gc[j] if j == p else 0 for j in range(n)])
            nop = nc.sync.nop(nofuse=True, hint="drain_split")
            wait_clock.add_sem_waits(nop.ins, ScopedClock({None: sub}))
        nc.sync.drain()
        nc.all_engine_barrier()
        popped = nc._tile_sem_poison_stack.pop()
        assert popped is self._sem_poison
        nc.clear_and_free_semaphores(list(self.sems.allocated().values()))
        nc.all_engine_barrier()

    tile.TileContext._drain_and_barrier = _drain_and_barrier
    tile.TileContext._ant_drain_patched = True


def _split_multi_waits(nc):
    """Hoist extra sync waits onto same-engine NoOps inserted right before
    the instruction (the engine stalls at the nop, semantics unchanged)."""
    import bass_rust
    import concourse.mybir as mybir

    k = 0
    for blk in nc.main_func.blocks:
        insts = blk.instructions
        need = False
        for ins in insts:
            si = ins.sync_info
            if si is not None and len(si.on_wait) > 1:
                need = True
                break
        if not need:
            continue
        out = []
        for ins in insts:
            si = ins.sync_info
            if (
                si is not None
                and len(si.on_wait) > 1
                and ins.engine != mybir.EngineType.Unassigned
            ):
                waits = list(si.on_wait)
                for w in waits[:-1]:
                    k += 1
                    nop = mybir.InstNoOp(
                        name=f"wsplit-{k}", ins=[], outs=[], engine=ins.engine
                    )
                    nop.sync_info = bass_rust.SyncInfo(on_wait=[w], on_update=[])
                    nc.register_instruction(nop, overwrite=True)
                    out.append(nop)
                ins.sync_info = bass_rust.SyncInfo(
                    on_wait=[waits[-1]], on_update=list(si.on_update)
                )
            out.append(ins)
        blk.instructions = out


def build_program(n_tokens):
    """Build the per-core Bass program (same for every core)."""
    import concourse.bass as bass
    import concourse.tile as tile
    import concourse.mybir as mybir

    _patch_tile_drain()

    dt = mybir.dt
    f32 = dt.float32
    f32r = dt.float32r
    bf16 = dt.bfloat16
    u32 = dt.uint32
    Alu = mybir.AluOpType
    Act = mybir.ActivationFunctionType

    nc = bass.Bass(trn_type="TRN2")

    k_d = nc.dram_tensor("k", [n_tokens, M * D], f32, kind="ExternalInput")
    pos_d = nc.dram_tensor("pos", [n_tokens, M * 4], f32, kind="ExternalInput")
    q_d = nc.dram_tensor("q", [n_tokens, D], f32, kind="ExternalInput")
    wsv_d = nc.dram_tensor("wsv", [128, 256], bf16, kind="ExternalInput")
    wpos_d = nc.dram_tensor("wpos", [64, 2048], bf16, kind="ExternalInput")
    woe_d = nc.dram_tensor("woe", [65, 64], bf16, kind="ExternalInput")
    w1e_d = nc.dram_tensor("w1e", [65, 256], f32r, kind="ExternalInput")
    w2s_d = nc.dram_tensor("w2s", [128, 128], f32r, kind="ExternalInput")
    b2c_d = nc.dram_tensor("b2c", [D], f32, kind="ExternalInput")
    ident_d = nc.dram_tensor("ident", [128, 128], bf16, kind="ExternalInput")
    ones_d = nc.dram_tensor("ones", [128], bf16, kind="ExternalInput")
    ones32_d = nc.dram_tensor("ones32", [128], f32r, kind="ExternalInput")
    ident32_d = nc.dram_tensor("ident32", [128, 128], f32r, kind="ExternalInput")
    out_d = nc.dram_tensor("out", [n_tokens, D], f32, kind="ExternalOutput")

    nchunks = n_tokens // CHUNK
    assert n_tokens % CHUNK == 0

    with tile.TileContext(nc) as tc:
        with (
            tc.tile_pool(name="singles", bufs=1) as singles,
            tc.tile_pool(name="kin", bufs=6) as kin,
            tc.tile_pool(name="small_in", bufs=8) as small_in,
            tc.tile_pool(name="ts", bufs=8) as tsp,
            tc.tile_pool(name="work", bufs=5) as work,
            tc.tile_pool(name="lnw", bufs=8) as lnw,
            tc.tile_pool(name="outp", bufs=6) as outp,
            tc.tile_pool(name="sp", bufs=2, space="PSUM") as sp_pool,
            tc.tile_pool(name="tp_ps", bufs=2, space="PSUM") as tp_ps,
            tc.tile_pool(name="tail_ps", bufs=2, space="PSUM") as tail_ps,
        ):
            # constants
            WSV = singles.tile([128, 256], bf16)
            WPOS = singles.tile([64, 2048], bf16)
            WOE = singles.tile([65, 64], bf16)
            W1E = singles.tile([65, 256], f32r)
            W2S = singles.tile([128, 128], f32r)
            B2B = singles.tile([128, D], f32)
            IDENT = singles.tile([128, 128], bf16)
            IDENT32 = singles.tile([128, 128], f32r)
            nc.sync.dma_start(out=IDENT32[:], in_=ident32_d[:])
            MAGIC = singles.tile([128, 1], u32)
            nc.vector.memset(MAGIC[:], RSQRT_MAGIC)
            OTSX = [
                singles.tile([65, 128], bf16, tag="otsx0", name="OTSX0"),
                singles.tile([65, 128], bf16, tag="otsx1", name="OTSX1"),
                singles.tile([65, 128], bf16, tag="otsx2", name="OTSX2"),
                singles.tile([65, 128], bf16, tag="otsx3", name="OTSX3"),
            ]
            HTSX = [
                singles.tile([65, 128], f32r, tag="htsx0", name="HTSX0"),
                singles.tile([65, 128], f32r, tag="htsx1", name="HTSX1"),
                singles.tile([65, 128], f32r, tag="htsx2", name="HTSX2"),
                singles.tile([65, 128], f32r, tag="htsx3", name="HTSX3"),
            ]
            for t in OTSX:
                nc.sync.dma_start(out=t[64:65, :], in_=ones_d[:])
            for t in HTSX:
                nc.sync.dma_start(out=t[64:65, :], in_=ones32_d[:])
            nc.sync.dma_start(out=WSV[:], in_=wsv_d[:])
            nc.sync.dma_start(out=WPOS[:], in_=wpos_d[:])
            nc.sync.dma_start(out=WOE[:], in_=woe_d[:])
            nc.sync.dma_start(out=W1E[:], in_=w1e_d[:])
            nc.sync.dma_start(out=W2S[:], in_=w2s_d[:])
            nc.sync.dma_start(out=IDENT[:], in_=ident_d[:])
            b2_bcast = bass.AP(
                tensor=b2c_d[:].tensor, offset=0, ap=[[0, 128], [1, D]]
            )
            nc.gpsimd.dma_start(out=B2B[:], in_=b2_bcast)

            def front(ci):
                n0 = ci * CHUNK

                KD = kin.tile([128, M * D], bf16, tag="kd")
                PD = small_in.tile([128, M * 4], bf16, tag="pd")
                QD = small_in.tile([128, D], f32, tag="qd")
                nc.gpsimd.dma_start(out=KD[:], in_=k_d[n0 : n0 + 128, :])
                nc.gpsimd.dma_start(out=PD[:], in_=pos_d[n0 : n0 + 128, :])
                nc.sync.dma_start(out=QD[:], in_=q_d[n0 : n0 + 128, :])

                # transposes of k (per m-pair) into PSUM, then to SBUF
                TP = tp_ps.tile([128, 1024], bf16, tag="tp")
                for j in range(8):
                    nc.tensor.transpose(
                        TP[:, 128 * j : 128 * (j + 1)],
                        KD[:, 128 * j : 128 * (j + 1)],
                        IDENT[:],
                    )
                TS = []
                for half in range(2):
                    TSh = tsp.tile([128, 512], bf16, tag="ts")
                    nc.scalar.copy(
                        out=TSh[:], in_=TP[:, 512 * half : 512 * (half + 1)]
                    )
                    TS.append(TSh)

                # pos transpose
                PT = tail_ps.tile([64, 128], bf16, tag="tlps")
                nc.tensor.transpose(PT[:], PD[:], IDENT[:])
                PTS = tsp.tile([64, 128], bf16, tag="pts")
                nc.scalar.copy(out=PTS[:], in_=PT[:])

                # S/v matmuls + pos accumulation; exp + e*v release PSUM
                EV = work.tile([128, 2, 2, 512], f32, tag="ev")
                for half in range(2):
                    SPh = sp_pool.tile([128, 4, 4, D], f32, tag="sp")
                    for jj in range(4):
                        nc.tensor.matmul(
                            SPh[:, jj, :, :],
                            TS[half][:, 128 * jj : 128 * (jj + 1)],
                            WSV[:],
                            start=(jj % 2 == 0),
                            stop=False,
                        )
                    for bi in range(2):
                        bank = 2 * half + bi
                        nc.tensor.matmul(
                            SPh[:, 2 * bi : 2 * bi + 2, :, :],
                            PTS[:],
                            WPOS[:, 512 * bank : 512 * (bank + 1)],
                            start=False,
                            stop=True,
                        )
                    nc.scalar.activation(
                        out=EV[:, 0, half, :],
                        in_=SPh[:, :, 0:2, :].rearrange("p j r d -> p d j r"),
                        func=Act.Exp,
                    )
                    nc.vector.tensor_mul(
                        EV[:, 1, half, :],
                        EV[:, 0, half, :],
                        SPh[:, :, 2:4, :].rearrange("p j r d -> p d j r"),
                    )
                return EV, QD

            def back1(ci, EV, QD):
                SM = lnw.tile([128, D], f32, tag="sm")
                OV = lnw.tile([128, D], f32, tag="ov")
                # sm = sum_m e via GpSimd tree (DVE relief)
                ER1 = lnw.tile([128, 512], f32, tag="er1")
                nc.gpsimd.tensor_add(ER1[:], EV[:, 0, 0, :], EV[:, 0, 1, :])
                ER2 = lnw.tile([128, 256], f32, tag="er2")
                e1v = ER1[:].rearrange("p (d m) -> p d m", d=D, m=8)
                nc.gpsimd.tensor_add(
                    ER2[:].rearrange("p (d m) -> p d m", d=D, m=4),
                    e1v[:, :, 0:4],
                    e1v[:, :, 4:8],
                )
                ER3 = lnw.tile([128, 128], f32, tag="er3")
                e2v = ER2[:].rearrange("p (d m) -> p d m", d=D, m=4)
                nc.gpsimd.tensor_add(
                    ER3[:].rearrange("p (d m) -> p d m", d=D, m=2),
                    e2v[:, :, 0:2],
                    e2v[:, :, 2:4],
                )
                e3v = ER3[:].rearrange("p (d m) -> p d m", d=D, m=2)
                nc.gpsimd.tensor_add(SM[:], e3v[:, :, 0], e3v[:, :, 1])
                nc.vector.tensor_reduce(
                    out=OV[:],
                    in_=EV[:, 1, :, :].rearrange("p h (d m) -> p d h m", d=D, m=8),
                    axis=mybir.AxisListType.XY,
                    op=Alu.add,
                )
                RC = lnw.tile([128, D], f32, tag="rc")
                nc.vector.reciprocal(out=RC[:], in_=SM[:])
                OA = lnw.tile([128, D], bf16, tag="oa")
                nc.gpsimd.tensor_mul(OA[:], OV[:], RC[:])
                return OA, QD

            def back2(ci, OA, QD):
                n0 = ci * CHUNK
                # Wo + residual
                OT = tail_ps.tile([64, 128], bf16, tag="tlps")
                nc.tensor.transpose(OT[:], OA[:], IDENT[:])
                OTS = OTSX[ci % 4]
                nc.scalar.copy(out=OTS[0:64, :], in_=OT[:])
                OO = tail_ps.tile([128, D], f32, tag="tlps")
                nc.tensor.matmul(OO[:], OTS[:], WOE[:], start=True, stop=True)
                R1 = lnw.tile([128, D], f32, tag="r1")
                nc.vector.tensor_add(R1[:], QD[:], OO[:])
                RB = lnw.tile([128, D], f32, tag="rb")
                nc.gpsimd.tensor_add(RB[:], R1[:], B2B[:])

                # LN2 stats (DVE) + Quake rsqrt chain (GpSimd)
                ST6 = lnw.tile([128, 6], f32, tag="st6")
                nc.vector.bn_stats(out=ST6[:], in_=R1[:])
                MV = lnw.tile([128, 2], f32, tag="mv")
                nc.vector.bn_aggr(out=MV[:], in_=ST6[:])
                VP = lnw.tile([128, 1], f32, tag="vp")
                YA = lnw.tile([128, 1], f32, tag="ya")
                YB = lnw.tile([128, 1], f32, tag="yb")
                nc.vector.tensor_scalar_add(VP[:], MV[:, 1:2], EPS)
                nc.vector.tensor_scalar(
                    out=YA[:].bitcast(u32),
                    in0=VP[:].bitcast(u32),
                    scalar1=1,
                    scalar2=None,
                    op0=Alu.logical_shift_right,
                )
                nc.gpsimd.tensor_tensor(
                    out=YA[:].bitcast(u32),
                    in0=MAGIC[:],
                    in1=YA[:].bitcast(u32),
                    op=Alu.subtract,
                )
                # Newton iterations: y <- y * (1.5 - 0.5 * vp * y^2)
                for _ in range(2):
                    nc.vector.tensor_scalar(
                        out=YB[:], in0=YA[:], scalar1=YA[:], scalar2=VP[:],
                        op0=Alu.mult, op1=Alu.mult,
                    )
                    nc.vector.tensor_scalar(
                        out=YB[:], in0=YB[:], scalar1=-0.5, scalar2=1.5,
                        op0=Alu.mult, op1=Alu.add,
                    )
                    nc.vector.tensor_scalar(
                        out=YA[:], in0=YA[:], scalar1=YB[:], scalar2=None,
                        op0=Alu.mult,
                    )
                CT = lnw.tile([128, D], f32r, tag="ct")
                nc.vector.tensor_scalar(
                    out=CT[:],
                    in0=R1[:],
                    scalar1=MV[:, 0:1],
                    scalar2=YA[:],
                    op0=Alu.subtract,
                    op1=Alu.mult,
                )
                return CT, RB

            def back3(ci, CT, RB):
                n0 = ci * CHUNK
                # MLP
                HT = tail_ps.tile([64, 128], f32r, tag="tlps")
                nc.tensor.transpose(HT[:], CT[:], IDENT32[:])
                HTS = HTSX[ci % 4]
                nc.scalar.copy(out=HTS[0:64, :], in_=HT[:])
                H1 = tail_ps.tile([128, F], f32, tag="tlps")
                nc.tensor.matmul(H1[:], HTS[:], W1E[:], start=True, stop=True)
                H1R = work.tile([128, F], f32r, tag="h1r")
                nc.scalar.activation(out=H1R[:], in_=H1[:], func=Act.Relu)
                HP = tail_ps.tile([128, 256], f32r, tag="tlps")
                nc.tensor.transpose(HP[:, 0:128], H1R[:, 0:128], IDENT32[:])
                nc.tensor.transpose(HP[:, 128:256], H1R[:, 128:256], IDENT32[:])
                H1TS = tsp.tile([128, 256], f32r, tag="h1ts")
                nc.scalar.copy(out=H1TS[:], in_=HP[:])
                H2 = tail_ps.tile([128, D], f32, tag="tlps")
                nc.tensor.matmul(
                    H2[:], H1TS[:, 0:128], W2S[:, 0:64], start=True, stop=False
                )
                nc.tensor.matmul(
                    H2[:], H1TS[:, 128:256], W2S[:, 64:128], start=False, stop=True
                )

                OUTT = outp.tile([128, D], f32, tag="outt")
                nc.vector.tensor_add(OUTT[:], H2[:], RB[:])
                nc.sync.dma_start(out=out_d[n0 : n0 + 128, :], in_=OUTT[:])

            L1, L2, L3 = 2, 1, 1
            pend1 = {}
            pend2 = {}
            pend3 = {}
            for ci in range(nchunks + L1 + L2 + L3):
                if ci < nchunks:
                    pend1[ci] = front(ci)
                c = ci - L1
                if 0 <= c < nchunks:
                    pend2[c] = back1(c, *pend1.pop(c))
                c = ci - L1 - L2
                if 0 <= c < nchunks:
                    pend3[c] = back2(c, *pend2.pop(c))
                c = ci - L1 - L2 - L3
                if 0 <= c < nchunks:
                    back3(c, *pend3.pop(c))

    _split_multi_waits(nc)
    return nc


def _get_program(n_tokens):
    if n_tokens not in _prog_cache:
        _prog_cache[n_tokens] = build_program(n_tokens)
    return _prog_cache[n_tokens]


def make_in_maps(inputs):
    """Shard full inputs into 8 per-core input maps."""
    w = _prep_weights(inputs)
    k = np.ascontiguousarray(inputs["k"])
    pos = np.ascontiguousarray(inputs["pos"])
    q = np.ascontiguousarray(inputs["q"])
    nt = k.shape[1]
    in_maps = []
    for b in range(B):
        in_maps.append(
            {
                "k": k[b].reshape(nt, M * D),
                "pos": pos[b].reshape(nt, M * 4),
                "q": q[b].reshape(nt, D),
                **w,
            }
        )
    return in_maps


LAST_EXEC_NS = None
LAST_RESULT = None


def _install_cc_probe():
    import subprocess
    import traceback

    import libneuronxla

    if getattr(libneuronxla, "_ant_probe", False):
        return
    shim = libneuronxla.neuronx_cc

    def loud(code, *a, **k):
        try:
            return shim(code, *a, **k)
        except subprocess.CalledProcessError as e:
            with open("/tmp/walrus_err.log", "w") as fh:
                fh.write(str(e.output))
            raise
        except BaseException:
            with open("/tmp/walrus_err.log", "w") as fh:
                fh.write(traceback.format_exc())
            raise

    libneuronxla.neuronx_cc = loud
    libneuronxla._ant_probe = True
    import concourse.bass2jax as b2j

    b2j.install_neuronx_cc_hook = lambda: None


def _ensure_ntff_hook():
    """Register the NTFF profiling hook if the image's antenv lacks it."""
    import sys
    import types

    try:
        from antenv.axon_hooks import get_axon_ntff_profile_hook  # noqa: F401

        return
    except ImportError:
        pass
    try:
        from trn_agent_boot.trn_boot import _ntff_profile_via_ctypes

        hook = _ntff_profile_via_ctypes("/opt/axon/libaxon_pjrt.so")
    except Exception:
        hook = None
    mod = types.ModuleType("antenv.axon_hooks")
    mod.get_axon_ntff_profile_hook = lambda: hook
    mod.set_axon_ntff_profile_hook = lambda h: None
    import antenv

    sys.modules["antenv.axon_hooks"] = mod
    antenv.axon_hooks = mod


def kernel(**inputs):
    global LAST_EXEC_NS, LAST_RESULT
    import os

    from concourse import bass_utils

    _install_cc_probe()
    trace = bool(int(os.environ.get("KERNEL_TRACE", "0")))
    if trace:
        _ensure_ntff_hook()
    nt = np.ascontiguousarray(inputs["k"]).shape[1]
    nc = _get_program(nt)
    in_maps = make_in_maps(inputs)
    res = bass_utils.run_bass_kernel_spmd(
        nc, in_maps, core_ids=list(range(B)), trace=trace
    )
    LAST_EXEC_NS = res.exec_time_ns
    LAST_RESULT = res
    out = np.stack([res.results[b]["out"].reshape(nt, D) for b in range(B)])
    return out.astype(np.float32)

